# revision 38
# baseline (speedup 1.0000x reference)
"""HINGCN (metapath GCN) Trainium2 kernel — 8-core SPMD, node-dim sharded.

Reference computation (N=8192, F=128, H=32, M=3 metapaths, C=16 classes):
    h1 = relu(A[m] @ (x @ W1[m]) + b1[m])          per metapath
    h2 = relu(A[m] @ (h1 @ W2[m]) + b2[m])
    e  = leaky_relu(h2 . a, 0.2); attn = softmax_m(e)
    out = sum_m attn[m] * h2[m];  logits = relu(out @ W_lin + b_lin)
    return log_softmax(logits)

Core k owns output rows u in [1024k, 1024k+1024). The adjacency row-blocks
stream in fp8-e4m3 (prescaled by N on the host so U[0,1/N) values stay in
e4m3 range; the 1/N descale folds into the activation `scale`). fp8 A+S
costs 1.4e-5 final rel err — the 8192-term A @ S contraction averages the
quantization noise away.

Schedule (the whole point of this kernel):
- DoubleRow fp8 matmuls contract TWO 128-row v-tiles per instruction
  (lhsT [128,2,H], rhs [128,2,512]) at 2 MACs/PE/cycle.
- A-tiles arrive as 1MB DMAs alternating across the two HWDGE queues
  (sync + scalar), which together sustain ~350GB/s; store DMAs ride the
  scalar queue and gather unpacks ride gpsimd SWDGE, so the A stream never
  head-of-line blocks behind a semaphore-waiting DMA.
- Metapath order is [2, 0, 1] in BOTH layers. m2's adjacency is loaded into
  SBUF-resident keeper tiles and reused by layer 2 (8MB/core of HBM traffic
  saved); its layer-1 matmuls run while the DMA stream races ahead into
  m0/m1.
- Each metapath's S2 = h1 @ W2 is quantized to fp8, stored contiguously,
  AllGathered (256KB) and unpacked the moment its h1 is done, so the later
  gathers hide under the layer-1/layer-2 streams. s1/s2f use an m-major
  layout so layer-2 metapath m only depends on gather m: L2-m2's operands
  (keepers + first gather) are resident as soon as the CC engine clears its
  startup barrier, and PE rarely waits on the last gather. (The CC engine
  cannot run collectives before ~72us — a device-barrier floor from host
  dispatch skew — which is why the gathers are deferred-tolerant by design.)
- Attention + head avoid transposes: one [97,128]-stationary matmul per
  u-tile against a block-diagonal [a | W_lin] + b_lin-row rhs yields e and
  h2 @ W_lin + b_lin for all 3 metapaths at once; softmax over m runs
  batched, and the attention normalization folds into the final relu scale.
"""

import numpy as np
import ml_dtypes
from contextlib import ExitStack

import concourse.bass as bass
import concourse.tile as tile
from concourse import bacc, mybir
from concourse.bass_utils import run_bass_kernel_spmd

NCORES = 8
N, F, H, M, C = 8192, 128, 32, 3, 16
UL = N // NCORES          # rows per core (1024)
VT = N // 128             # 128-row v-tiles (64)
VT2 = VT // 2             # v-tile pairs (32) — one DoubleRow matmul each
TQ = 4                    # v-tile pairs per A-stream DMA (1MB transfers)
NQ = VT2 // TQ            # A-stream DMAs per metapath per layer (8)
UT = UL // 128            # 128-row u-tiles per core (8)
NSTRIP = UL // 512        # 512-wide psum strips per core (2)
ALPHA = 0.2
SC = float(N)             # host prescale on A; descaled in activations

BF = mybir.dt.bfloat16
F32 = mybir.dt.float32
FP8 = mybir.dt.float8e4
AX = mybir.AxisListType.X
AF = mybir.ActivationFunctionType
OP = mybir.AluOpType
DR = mybir.MatmulPerfMode.DoubleRow

MH = M * H                # 96
EW = C + 1                # e + logits columns per metapath (17)
SB = VT * H               # per-metapath S-matrix columns (2048)
MORD = [M - 1] + list(range(M - 1))   # [2, 0, 1]


def build_kernel_body(nc, tc, ctx, t_in, out_dram):
    xt, at, w1, w2, b1t, b2t, egw = (
        t_in["xt"], t_in["at"], t_in["w1"], t_in["w2"],
        t_in["b1t"], t_in["b2t"], t_in["egw"])

    const = ctx.enter_context(tc.tile_pool(name="const", bufs=1))
    sbuf = ctx.enter_context(tc.tile_pool(name="sbuf", bufs=2))
    atp = ctx.enter_context(tc.tile_pool(name="atp", bufs=4))
    psA = ctx.enter_context(tc.tile_pool(name="psA", bufs=4, space="PSUM"))
    psW = ctx.enter_context(tc.tile_pool(name="psW", bufs=3, space="PSUM"))
    dram = ctx.enter_context(tc.tile_pool(name="dram", bufs=1, space="DRAM"))

    # ---- constants / parameters in SBUF ----
    # xt + weights lead the scalar queue; the sync queue starts directly
    # with the m2 keeper tiles so the PE's first DoubleRow matmuls (which
    # chase S1 production) never wait on a queue-position artifact
    xt_sb = const.tile([128, N], BF)
    nc.scalar.dma_start(xt_sb[:, 0:N // 2], xt[:, 0:N // 2])
    nc.scalar.dma_start(xt_sb[:, N // 2:N], xt[:, N // 2:N])
    w1_sb = const.tile([128, MH], BF)
    nc.scalar.dma_start(w1_sb[:], w1[:])
    w2_sb = const.tile([H, MH], BF)
    nc.scalar.dma_start(w2_sb[:], w2[:])
    b1t_sb = const.tile([H, M], F32)
    nc.scalar.dma_start(b1t_sb[:], b1t[:])
    b2t_sb = const.tile([H, M], F32)
    nc.scalar.dma_start(b2t_sb[:], b2t[:])
    egw_sb = const.tile([M * H + 1, M * EW], BF)
    nc.scalar.dma_start(egw_sb[:], egw[:])

    # S matrices in m-major v-partition layout: col = m*SB + vt*H + h
    s1_sb = const.tile([128, M * SB], FP8)
    s2f_sb = const.tile([128, M * SB], FP8)
    h1t_sb = [const.tile([H, UL], BF, name=f"h1t_{m}") for m in range(M)]
    h2all_sb = const.tile([M * H + 1, UL], BF)      # [(m,h) rows + ones row]
    nc.vector.memset(h2all_sb[M * H:M * H + 1, :], 1.0)
    s2st_sb = [const.tile([128, UT * H], FP8, name=f"s2st_{m}")
               for m in range(M)]

    # keeper tiles: metapath 2's adjacency stays SBUF-resident from layer 1
    # so layer 2 re-reads it without touching HBM (8MB/core saved)
    keep = [const.tile([128, TQ * 2 * UL], FP8, name=f"keep{q}")
            for q in range(NQ)]

    # touch Ln once now so the scalar engine's activation table for the
    # epilogue's log_softmax is already loaded (saves a 1.3us mid-epilogue
    # ACT_TABLE_LOAD); ln(1.0) = 0 keeps the sim's NaN checks happy
    lnw = sbuf.tile([1, 1], F32, tag="lnw", name="lnw")
    nc.vector.memset(lnw[:], 1.0)
    lnwo = sbuf.tile([1, 1], F32, tag="lnwo", name="lnwo")
    nc.scalar.activation(lnwo[:], lnw[:], AF.Ln)

    # ---- S1 = x @ W1 (all metapaths per matmul), quantize to fp8 ----
    s1v = s1_sb[:].rearrange("p (m vt h) -> p m vt h", m=M, vt=VT)
    for vt in range(VT):
        ps1 = psW.tile([128, MH], F32, tag="w96", name="ps1")
        nc.tensor.matmul(ps1[:], xt_sb[:, vt * 128:(vt + 1) * 128], w1_sb[:],
                         start=True, stop=True)
        nc.vector.tensor_copy(s1v[:, :, vt, :],
                              ps1[:].rearrange("p (m h) -> p m h", m=M))

    # ---- GCN layer stream: acc[m,s] = sum_t S[t].T @ AT[m,t] (DoubleRow) ----
    def stream_layer(s_sb, layer):
        sv = s_sb[:].rearrange("p (m t two h) -> p m t two h",
                               m=M, t=VT2, two=2)
        for m in MORD:
            acc = [psA.tile([H, 512], F32, tag="acc", name=f"acc{layer}{m}{s}")
                   for s in range(NSTRIP)]
            for q in range(NQ):
                if m == M - 1:
                    att = keep[q]
                    if layer == 0:
                        nc.sync.dma_start(att[:], at[m, q])
                else:
                    att = atp.tile([128, TQ * 2 * UL], FP8, tag="at",
                                   name="att")
                    deng = nc.sync if q % 2 == 0 else nc.scalar
                    deng.dma_start(att[:], at[m, q])
                av = att[:].rearrange("p (tt two u) -> p tt two u",
                                      tt=TQ, two=2)
                for tt in range(TQ):
                    t = q * TQ + tt
                    lhs = sv[:, m, t, :, :]
                    for s in range(NSTRIP):
                        nc.tensor.matmul(
                            acc[s][:], lhs,
                            av[:, tt, :, s * 512:(s + 1) * 512],
                            start=(t == 0), stop=(t == VT2 - 1),
                            perf_mode=DR)
            yield m, acc

    # ---- layer 1; each metapath's S2 = h1 @ W2 is quantized, stored,
    # AllGathered and unpacked as soon as that metapath's h1 is ready, so
    # the exchanges pipeline under the rest of the layer-1 stream ----
    s2loc = [dram.tile([128, UT * H], FP8, name=f"s2loc{m}")
             for m in range(M)]
    s2full = [dram.tile([NCORES * 128, UT * H], FP8, addr_space="Shared",
                        name=f"s2full{m}") for m in range(M)]
    def unpack_s2(m):
        # global vt = 8r + ut, so gathered block r of metapath m lands at
        # s2f columns [m*SB + 256r, m*SB + 256r + 256) as one strided copy
        nc.gpsimd.dma_start(
            s2f_sb[:, m * SB:(m + 1) * SB].rearrange("p (r w) -> p r w",
                                                     r=NCORES),
            s2full[m][:].rearrange("(r p) w -> p r w", p=128))

    for m, acc in stream_layer(s1_sb, 0):
        for s in range(NSTRIP):
            nc.scalar.activation(h1t_sb[m][:, s * 512:(s + 1) * 512], acc[s][:],
                                 AF.Relu, bias=b1t_sb[:, m:m + 1], scale=1.0 / SC)
        for ut in range(UT):
            ps2 = psW.tile([128, MH], F32, tag="w96", name="ps2")
            nc.tensor.matmul(ps2[:, 0:H], h1t_sb[m][:, ut * 128:(ut + 1) * 128],
                             w2_sb[:, m * H:(m + 1) * H], start=True, stop=True)
            nc.vector.tensor_copy(s2st_sb[m][:, ut * H:(ut + 1) * H],
                                  ps2[:, 0:H])
        nc.scalar.dma_start(s2loc[m][:], s2st_sb[m][:])
        nc.gpsimd.collective_compute(
            "AllGather", OP.bypass, replica_groups=[list(range(NCORES))],
            ins=[s2loc[m][:].opt()], outs=[s2full[m][:].opt()])
        unpack_s2(m)

    # ---- layer 2 -> h2 (bf16, with a shared ones row for the bias trick).
    # L2-m2 starts the instant layer 1 ends: its adjacency (keepers) and its
    # s2f block (first gather) are both already on-chip.
    for m, acc in stream_layer(s2f_sb, 1):
        for s in range(NSTRIP):
            nc.scalar.activation(
                h2all_sb[m * H:(m + 1) * H, s * 512:(s + 1) * 512], acc[s][:],
                AF.Relu, bias=b2t_sb[:, m:m + 1], scale=1.0 / SC)

    # ---- metapath attention + linear head, batched over all 8 u-tiles ----
    # One matmul per u-tile: [97,128] h2-stationary x [97,51] block-diag
    # [a | W_lin] + b_lin row -> per metapath: e column + 16 logit columns.
    eg_sb = const.tile([128, UT * M * EW], F32)     # [(ut),(m),(e|g)]
    for ut in range(UT):
        pse = psW.tile([128, MH], F32, tag="w96", name="pse")
        nc.tensor.matmul(pse[:, 0:M * EW],
                         h2all_sb[:, ut * 128:(ut + 1) * 128], egw_sb[:],
                         start=True, stop=True)
        nc.vector.tensor_copy(eg_sb[:, ut * M * EW:(ut + 1) * M * EW],
                              pse[:, 0:M * EW])

    egv = eg_sb[:].rearrange("p (ut m w) -> p ut m w", ut=UT, m=M)
    etv = egv[:, :, :, 0]                                       # [128, ut, m]
    eta = sbuf.tile([128, UT * M], F32, tag="eta", name="eta")
    nc.vector.tensor_scalar_mul(eta[:].rearrange("p (ut m) -> p ut m", m=M),
                                etv, ALPHA)
    etl = sbuf.tile([128, UT * M], F32, tag="etl", name="etl")
    nc.vector.tensor_max(etl[:].rearrange("p (ut m) -> p ut m", m=M),
                         etv, eta[:].rearrange("p (ut m) -> p ut m", m=M))
    etlv = etl[:].rearrange("p (ut m) -> p ut m", m=M)
    nmx = sbuf.tile([128, UT], F32, tag="nmx", name="nmx")
    nc.vector.reduce_max(nmx[:], etlv, axis=AX, negate=True)    # -max_m
    exm = sbuf.tile([128, M * UT], F32, tag="exm", name="exm")  # [m, ut]
    for m in range(M):
        nc.vector.tensor_add(exm[:, m * UT:(m + 1) * UT], etlv[:, :, m], nmx[:])
    exe = sbuf.tile([128, M * UT], F32, tag="exe", name="exe")
    nc.scalar.activation(exe[:], exm[:], AF.Exp)
    s01 = sbuf.tile([128, UT], F32, tag="s01", name="s01")
    nc.vector.tensor_add(s01[:], exe[:, 0:UT], exe[:, UT:2 * UT])
    ssum = sbuf.tile([128, UT], F32, tag="ssum", name="ssum")
    nc.vector.tensor_add(ssum[:], s01[:], exe[:, 2 * UT:3 * UT])
    rs = sbuf.tile([128, UT], F32, tag="rs", name="rs")
    nc.vector.reciprocal(rs[:], ssum[:])

    # out = sum_m softmax_m * (h2[m] @ W_lin + b_lin); the 1/sum(exp)
    # normalization folds into the final relu's scale
    lgr = const.tile([128, UT * C], F32)
    for ut in range(UT):
        g = [egv[:, ut, m, 1:EW] for m in range(M)]
        e_ = [exe[:, m * UT + ut: m * UT + ut + 1] for m in range(M)]
        t0 = sbuf.tile([128, C], F32, tag="t0", name="t0")
        nc.vector.tensor_scalar_mul(t0[:], g[0], e_[0])
        t1 = sbuf.tile([128, C], F32, tag="t1", name="t1")
        nc.vector.scalar_tensor_tensor(t1[:], g[1], e_[1], t0[:],
                                       op0=OP.mult, op1=OP.add)
        t2 = sbuf.tile([128, C], F32, tag="t2", name="t2")
        nc.vector.scalar_tensor_tensor(t2[:], g[2], e_[2], t1[:],
                                       op0=OP.mult, op1=OP.add)
        nc.scalar.activation(lgr[:, ut * C:(ut + 1) * C], t2[:], AF.Relu,
                             scale=rs[:, ut:ut + 1])

    # log_softmax over classes, batched; logits are relu'd (small, >=0)
    # so exp needs no max-shift: out = lgr - ln(sum(exp(lgr)))
    exs = sbuf.tile([128, UT * C], F32, tag="exs", name="exs")
    nc.scalar.activation(exs[:], lgr[:], AF.Exp)
    sm = sbuf.tile([128, UT], F32, tag="sm", name="sm")
    nc.vector.reduce_sum(sm[:], exs[:].rearrange("p (ut c) -> p ut c", c=C),
                         axis=AX)
    lssum = sbuf.tile([128, UT], F32, tag="lssum", name="lssum")
    nc.scalar.activation(lssum[:], sm[:], AF.Ln)
    fin = sbuf.tile([128, UT * C], F32, tag="fin", name="fin")
    for ut in range(UT):
        nc.vector.tensor_scalar_sub(fin[:, ut * C:(ut + 1) * C],
                                    lgr[:, ut * C:(ut + 1) * C],
                                    lssum[:, ut:ut + 1])
    nc.scalar.dma_start(
        out_dram[:].rearrange("(ut p) c -> p ut c", p=128),
        fin[:].rearrange("p (ut c) -> p ut c", c=C))


_CACHED = {}


def build():
    if "nc" in _CACHED:
        return _CACHED["nc"]
    nc = bacc.Bacc("TRN2", target_bir_lowering=False, debug=False,
                   num_devices=NCORES)
    t_in = {
        "xt": nc.dram_tensor("xt", [128, N], BF, kind="ExternalInput").ap(),
        "at": nc.dram_tensor("at", [M, NQ, 128, TQ * 2 * UL], FP8,
                             kind="ExternalInput").ap(),
        "w1": nc.dram_tensor("w1", [128, MH], BF, kind="ExternalInput").ap(),
        "w2": nc.dram_tensor("w2", [H, MH], BF, kind="ExternalInput").ap(),
        "b1t": nc.dram_tensor("b1t", [H, M], F32, kind="ExternalInput").ap(),
        "b2t": nc.dram_tensor("b2t", [H, M], F32, kind="ExternalInput").ap(),
        "egw": nc.dram_tensor("egw", [M * H + 1, M * EW], BF,
                              kind="ExternalInput").ap(),
    }
    out_dram = nc.dram_tensor("out", [UL, C], F32, kind="ExternalOutput").ap()
    with tile.TileContext(nc) as tc, ExitStack() as ctx:
        build_kernel_body(nc, tc, ctx, t_in, out_dram)
    nc.compile()
    _CACHED["nc"] = nc
    return nc


def _bf16(x):
    """Fast f32 -> bf16 with round-to-nearest-even via integer ops."""
    x = np.ascontiguousarray(x, dtype=np.float32)
    u = x.view(np.uint32)
    r = ((u + 0x7FFF + ((u >> 16) & 1)) >> 16).astype(np.uint16)
    return r.view(ml_dtypes.bfloat16)


def make_in_maps(x, adjs, W1, b1, W2, b2, a, W_lin, b_lin):
    xt = np.ascontiguousarray(_bf16(x).T)                       # [128, N]
    w1 = np.ascontiguousarray(_bf16(W1).transpose(1, 0, 2)).reshape(128, MH)
    w2 = np.ascontiguousarray(_bf16(W2).transpose(1, 0, 2)).reshape(H, MH)
    b1t = np.ascontiguousarray(b1.T, dtype=np.float32)          # [H, M]
    b2t = np.ascontiguousarray(b2.T, dtype=np.float32)
    # block-diagonal [a | W_lin] per metapath + shared b_lin row, [97, 51]
    egw = np.zeros((M * H + 1, M * EW), dtype=np.float32)
    for m in range(M):
        egw[m * H:(m + 1) * H, m * EW] = a
        egw[m * H:(m + 1) * H, m * EW + 1:(m + 1) * EW] = W_lin
        egw[M * H, m * EW + 1:(m + 1) * EW] = b_lin
    egw = _bf16(egw)
    # fp8 adjacency, prescaled by N so U[0,1/N) values land in e4m3 range
    adjs_q = (np.asarray(adjs, dtype=np.float32) * SC).astype(
        ml_dtypes.float8_e4m3)                                  # [M, N, N]
    in_maps = []
    for k in range(NCORES):
        # at[m, q, p, (tt, i, u)] = A[m][v = 2048q + 256tt + 128i + p,
        #                                 rows_k[u]] (transposed row-block)
        blk = adjs_q[:, k * UL:(k + 1) * UL, :]                 # [M, UL, N]
        atk = np.ascontiguousarray(
            blk.transpose(0, 2, 1)                              # [M, N(v), UL]
            .reshape(M, NQ, TQ, 2, 128, UL)
            .transpose(0, 1, 4, 2, 3, 5)                        # m q p tt i u
            .reshape(M, NQ, 128, TQ * 2 * UL))
        in_maps.append({"xt": xt, "at": atk, "w1": w1, "w2": w2,
                        "b1t": b1t, "b2t": b2t, "egw": egw})
    return in_maps


def kernel(x, adjs, W1, b1, W2, b2, a, W_lin, b_lin, _trace=False):
    nc = build()
    in_maps = make_in_maps(x, adjs, W1, b1, W2, b2, a, W_lin, b_lin)
    res = run_bass_kernel_spmd(nc, in_maps, core_ids=list(range(NCORES)),
                               trace=_trace)
    out = np.concatenate([res.results[k]["out"] for k in range(NCORES)], axis=0)
    if _trace:
        kernel.last_result = res
    return out
